# revision 6
# baseline (speedup 1.0000x reference)
"""Biased multi-head cross-attention on 8 TRN2 NeuronCores.

Math (per reference): q/k/v linear projections, scores = q@k^T/sqrt(hd) + bias,
softmax over source dim, attn = probs@v, then the "faithful" reshape
(B,H,T,hd)->(B,T,E) WITHOUT transposing heads back, followed by the out
projection. That reshape makes output rows [h*64,(h+1)*64) of each batch
depend only on head h, so the whole problem is embarrassingly parallel over
(batch, head): core c owns batch c//2 and heads (c%2)*8..(c%2)*8+8, computes
its 512 output rows, and the host concatenates. No collectives.

v2 structure (PE was the baseline bottleneck at ~400us busy):
  - bias is DMA'd with an inline f32->bf16 cast (SWDGE) in [128t, 2048s]
    fully-contiguous 1MB tiles; the PE transposes bf16 bias chunks at
    1 cyc/row (vs 2 for f32) straight into the score PSUM bank, then the
    K^T.Q matmul accumulates on top (single accumulation group per bank).
  - q/k/v projections pack BOTH heads of a pair into one M=128 stationary
    (half the matmul streams of the per-head version).
  - attn^T accumulates in PSUM with the ones-column denominator trick;
    evacuation normalizes in place (reciprocal + gpsimd partition
    broadcast + DVE multiply) into a resident SBUF atT buffer.
  - out projection runs straight from SBUF: the faithful reshape is a
    stride-16 stationary access pattern on atT (no DRAM scratch), with
    Wo^T stored K=64-aligned on partitions 0-63 as [64, 16, E].
"""

import sys

for _p in ("/opt/trn_rl_repo", "/root/.axon_site/_ro/trn_rl_repo"):
    if _p not in sys.path:
        sys.path.insert(0, _p)

import numpy as np

B, T, S, E = 4, 1024, 2048, 1024
H, HD = 16, 64
NCORES = 8
NH = 8          # heads per core
P = 128
TB = T // 512   # t-blocks of 512
NSI = S // P    # s-tiles of 128
scaling = HD ** -0.5

_cache = {}


def _build_nc():
    import concourse.mybir as mybir
    import concourse.tile as tile
    from concourse import bacc
    from concourse.masks import make_identity

    f32 = mybir.dt.float32
    bf16 = mybir.dt.bfloat16
    Exp = mybir.ActivationFunctionType.Exp
    Ident = mybir.ActivationFunctionType.Identity

    nc = bacc.Bacc(None, target_bir_lowering=False)

    hs_p = nc.declare_dram_parameter("hs", [T, E], f32, isOutput=False)
    kv_p = nc.declare_dram_parameter("kv", [S, E], f32, isOutput=False)
    bias_p = nc.declare_dram_parameter("bias", [NH, T, S], f32, isOutput=False)
    Wq_p = nc.declare_dram_parameter("Wq", [NH * HD, E], f32, isOutput=False)
    Wk_p = nc.declare_dram_parameter("Wk", [NH * HD, E], f32, isOutput=False)
    Wv_p = nc.declare_dram_parameter("Wv", [NH * HD, E], f32, isOutput=False)
    Wo_p = nc.declare_dram_parameter("Wo", [E, E], f32, isOutput=False)
    bq_p = nc.declare_dram_parameter("bq", [NH * HD], f32, isOutput=False)
    bk_p = nc.declare_dram_parameter("bk", [NH * HD], f32, isOutput=False)
    bv_p = nc.declare_dram_parameter("bv", [NH * HD], f32, isOutput=False)
    bo_p = nc.declare_dram_parameter("bo", [E], f32, isOutput=False)
    out_p = nc.declare_dram_parameter("out", [NH * HD, E], f32, isOutput=True)

    halves = (slice(0, HD), slice(HD, P))

    with tile.TileContext(nc) as tc:
        with tc.tile_pool(name="statics", bufs=1) as statics:
            id_f32 = statics.tile([P, P], f32)
            make_identity(nc, id_f32[:])
            id_bf = statics.tile([P, P], bf16)
            make_identity(nc, id_bf[:])

            # Wo^T with e' (the contraction index) K=64-aligned: [64, 16, E],
            # WoT64[d, j, e] = Wo[e, j*64+d]
            WoT64 = statics.tile([HD, 16, E], bf16)
            bq_s = statics.tile([P, NH // 2], f32)   # bq_s[p,hp]=0.125*bq[hp*128+p]
            bk_s = statics.tile([P, NH // 2], f32)
            bv_rep = statics.tile([P, NH // 2, P], f32)
            bo_rep = statics.tile([P, E], f32)
            # normalized attn^T, resident: atT[d, hp, g, t] = attn[2hp+g, t, d]/den
            atT = statics.tile([HD, NH // 2, 2, T], bf16)

            with tc.tile_pool(name="big", bufs=1) as big, \
                 tc.tile_pool(name="stage", bufs=2) as stage_pool, \
                 tc.tile_pool(name="pairw", bufs=2) as pairw, \
                 tc.tile_pool(name="pairqkv", bufs=2) as pairqkv, \
                 tc.tile_pool(name="nrm", bufs=2) as nrm_pool, \
                 tc.tile_pool(name="bias", bufs=8) as bias_pool, \
                 tc.tile_pool(name="pt", bufs=8) as pt_pool, \
                 tc.tile_pool(name="tp", bufs=1, space="PSUM") as tp_pool, \
                 tc.tile_pool(name="proj", bufs=2, space="PSUM") as proj_pool, \
                 tc.tile_pool(name="sc", bufs=3, space="PSUM") as sc_pool, \
                 tc.tile_pool(name="at", bufs=2, space="PSUM") as at_pool:

                # ---- small vectors ----
                nc.sync.dma_start(bq_s[:], bq_p.rearrange("(hp p) -> p hp", p=P))
                nc.vector.tensor_scalar_mul(bq_s[:], bq_s[:], scaling)
                nc.sync.dma_start(bk_s[:], bk_p.rearrange("(hp p) -> p hp", p=P))
                bv_row = stage_pool.tile([1, NH * HD], f32, tag="vec")
                nc.sync.dma_start(bv_row[:], bv_p[None, :])
                for hp in range(NH // 2):
                    nc.gpsimd.partition_broadcast(
                        bv_rep[:, hp, :], bv_row[0:1, hp * P:(hp + 1) * P])
                bo_row = stage_pool.tile([1, E], f32, tag="vec")
                nc.sync.dma_start(bo_row[:], bo_p[None, :])
                nc.gpsimd.partition_broadcast(bo_rep[:], bo_row[0:1, :])

                def transpose_in(dst, src_ap, nrow_tiles, row0=0):
                    # src [nrow_tiles*128, E] f32 -> dst[:, j, i*128...] bf16
                    # (E on partitions), batched evacuation 4 chunks per bank
                    for i in range(nrow_tiles):
                        st = stage_pool.tile([P, E], f32, tag="st")
                        r = row0 + i * P
                        nc.sync.dma_start(st[:], src_ap[r:r + P, :])
                        for jb in range(2):
                            tp = tp_pool.tile([P, 4, P], f32, tag="tp")
                            for a in range(4):
                                nc.tensor.matmul(
                                    tp[:, a, :], st[:, (jb * 4 + a) * P:(jb * 4 + a + 1) * P],
                                    id_f32[:], is_transpose=True,
                                    start=(a == 0), stop=(a == 3))
                            nc.vector.tensor_copy(
                                dst[:, jb * 4:(jb + 1) * 4, i * P:(i + 1) * P], tp[:])

                hsT = big.tile([P, 8, T], bf16)
                kvT = big.tile([P, 8, S], bf16)
                transpose_in(hsT, hs_p, T // P)
                transpose_in(kvT, kv_p, S // P)

                # Wo: transpose 64-col chunks so e' lands on partitions 0-63
                for i in range(E // P):
                    st = stage_pool.tile([P, E], f32, tag="st")
                    nc.sync.dma_start(st[:], Wo_p[i * P:(i + 1) * P, :])
                    for jb in range(4):
                        tp = tp_pool.tile([P, 4, P], f32, tag="tp")
                        for a in range(4):
                            c = jb * 4 + a
                            nc.tensor.matmul(
                                tp[0:HD, a, :], st[:, c * HD:(c + 1) * HD],
                                id_f32[:], is_transpose=True,
                                start=(a == 0), stop=(a == 3))
                        nc.vector.tensor_copy(
                            WoT64[:, jb * 4:(jb + 1) * 4, i * P:(i + 1) * P],
                            tp[0:HD, :, :])

                for hp in range(NH // 2):
                    # ---- this pair's weight slices, transposed ----
                    WqTp = pairw.tile([P, 8, P], bf16, tag="wq")
                    WkTp = pairw.tile([P, 8, P], bf16, tag="wk")
                    WvTp = pairw.tile([P, 8, P], bf16, tag="wv")
                    for dst, src in ((WqTp, Wq_p), (WkTp, Wk_p), (WvTp, Wv_p)):
                        st = stage_pool.tile([P, E], f32, tag="st")
                        nc.sync.dma_start(st[:], src[hp * P:(hp + 1) * P, :])
                        for jb in range(2):
                            tp = tp_pool.tile([P, 4, P], f32, tag="tp")
                            for a in range(4):
                                nc.tensor.matmul(
                                    tp[:, a, :], st[:, (jb * 4 + a) * P:(jb * 4 + a + 1) * P],
                                    id_f32[:], is_transpose=True,
                                    start=(a == 0), stop=(a == 3))
                            nc.vector.tensor_copy(
                                dst[:, jb * 4:(jb + 1) * 4, :], tp[:])

                    # ---- projections for the pair (both heads per matmul) ----
                    qTp = pairqkv.tile([P, T], bf16, tag="qTp")
                    kTp = pairqkv.tile([P, S], bf16, tag="kTp")
                    v_aug = pairqkv.tile([P, 2, NSI, HD + 1], bf16, tag="vaug")
                    nc.any.memset(v_aug[:, :, :, HD:HD + 1], 1.0)
                    for tb in range(TB):
                        ps = proj_pool.tile([P, 512], f32, tag="proj")
                        for j in range(8):
                            nc.tensor.matmul(
                                ps[:], WqTp[:, j, :],
                                hsT[:, j, tb * 512:(tb + 1) * 512],
                                start=(j == 0), stop=(j == 7))
                        nc.scalar.activation(
                            qTp[:, tb * 512:(tb + 1) * 512], ps[:], Ident,
                            bias=bq_s[:, hp:hp + 1], scale=scaling)
                    for sb in range(S // 512):
                        ps = proj_pool.tile([P, 512], f32, tag="proj")
                        for j in range(8):
                            nc.tensor.matmul(
                                ps[:], WkTp[:, j, :],
                                kvT[:, j, sb * 512:(sb + 1) * 512],
                                start=(j == 0), stop=(j == 7))
                        nc.scalar.activation(
                            kTp[:, sb * 512:(sb + 1) * 512], ps[:], Ident,
                            bias=bk_s[:, hp:hp + 1])
                    for si in range(NSI):
                        ps = proj_pool.tile([P, P], f32, tag="proj")
                        for j in range(8):
                            nc.tensor.matmul(
                                ps[:], kvT[:, j, si * P:(si + 1) * P], WvTp[:, j, :],
                                start=(j == 0), stop=(j == 7))
                        nc.vector.tensor_tensor(
                            v_aug[:, :, si, 0:HD],
                            ps.rearrange("p (g d) -> p g d", g=2),
                            bv_rep[:, hp, :].rearrange("p (g d) -> p g d", g=2),
                            mybir.AluOpType.add)

                    # ---- scores + softmax + attn, per head of the pair ----
                    for g in range(2):
                        hl = hp * 2 + g
                        gsl = halves[g]
                        for tb in range(TB):
                            tsl = slice(tb * 512, (tb + 1) * 512)
                            # 4 t-tiles of bias, cast to bf16 during DMA
                            bts = []
                            for a in range(4):
                                bst = bias_pool.tile(
                                    [P, NSI, P], bf16, tag="bst",
                                    name=f"bst_{hl}_{tb}_{a}")
                                r = tb * 512 + a * P
                                nc.gpsimd.dma_start(
                                    bst[:], bias_p[hl, r:r + P, :]
                                    .rearrange("p (si c) -> p si c", c=P))
                                bts.append(bst)
                            at_ps = at_pool.tile([HD + 1, 512], f32, tag="at",
                                                 name=f"at_{hl}_{tb}")
                            pend = []
                            for si in range(NSI):
                                sc_ps = sc_pool.tile([P, 512], f32, tag="sc")
                                for a in range(4):
                                    # regular matmul bias_chunk^T @ I == bias^T
                                    # (bf16 in, f32 PSUM out; is_transpose
                                    # would force a bf16 PSUM tile)
                                    nc.tensor.matmul(
                                        sc_ps[:, a * P:(a + 1) * P],
                                        bts[a][:, si, :],
                                        id_bf[:],
                                        start=(a == 0), stop=False)
                                nc.tensor.matmul(
                                    sc_ps[:], kTp[gsl, si * P:(si + 1) * P],
                                    qTp[gsl, tsl], start=False, stop=True)
                                pt = pt_pool.tile([P, 512], bf16, tag="pt")
                                nc.scalar.activation(pt[:], sc_ps[:], Exp)
                                pend.append((si, pt))
                                # drain attn matmuls a couple of si behind so
                                # PE's strict FIFO never waits on ACT's exp
                                while len(pend) > 2:
                                    si2, pt2 = pend.pop(0)
                                    nc.tensor.matmul(
                                        at_ps[:], v_aug[:, g, si2, :], pt2[:],
                                        start=(si2 == 0), stop=(si2 == NSI - 1))
                            for si2, pt2 in pend:
                                nc.tensor.matmul(
                                    at_ps[:], v_aug[:, g, si2, :], pt2[:],
                                    start=(si2 == 0), stop=(si2 == NSI - 1))
                            # evacuate + normalize: atT = attn^T * (1/den).
                            # partition_broadcast only reads partition 0 on
                            # HW, so DMA the reciprocal row from partition 64
                            # down to partition 0 first.
                            den = nrm_pool.tile([HD + 1, 512], f32, tag="den")
                            nc.vector.reciprocal(
                                den[HD:HD + 1, :], at_ps[HD:HD + 1, :])
                            rec0 = nrm_pool.tile([1, 512], f32, tag="rec0")
                            nc.sync.dma_start(rec0[:], den[HD:HD + 1, :])
                            recb = nrm_pool.tile([HD, 512], f32, tag="recb")
                            nc.gpsimd.partition_broadcast(
                                recb[:], rec0[0:1, :])
                            nc.vector.tensor_tensor(
                                atT[:, hp, g, tsl], at_ps[0:HD, :], recb[:],
                                mybir.AluOpType.mult)

            # ---------------- phase 3: out projection from SBUF ----------------
            # out[hp*128 + g*64 + tt, e] = sum_j sum_d atT[d, hp, g, 16tt+j]
            #                              * WoT64[d, j, e]
            with tc.tile_pool(name="p3_sb", bufs=2) as p3_sb, \
                 tc.tile_pool(name="p3_o", bufs=2, space="PSUM") as o_pool:
                for hp in range(NH // 2):
                    for n in range(2):
                        po = o_pool.tile([P, 512], f32, tag="po")
                        for j in range(16):
                            nc.tensor.matmul(
                                po[:],
                                atT[:, hp, :, j::16],
                                WoT64[:, j, n * 512:(n + 1) * 512],
                                start=(j == 0), stop=(j == 15))
                        ob = p3_sb.tile([P, 512], f32, tag="ob")
                        nc.vector.tensor_tensor(
                            ob[:], po[:], bo_rep[:, n * 512:(n + 1) * 512],
                            mybir.AluOpType.add)
                        nc.sync.dma_start(
                            out_p[hp * P:(hp + 1) * P, n * 512:(n + 1) * 512],
                            ob[:])

    nc.compile()
    return nc


def get_nc():
    if "nc" not in _cache:
        _cache["nc"] = _build_nc()
    return _cache["nc"]


def make_in_maps(inputs):
    f = lambda x: np.asarray(x, dtype=np.float32)
    hs = f(inputs["hidden_states"])
    kv = f(inputs["key_value_states"])
    bias = f(inputs["bias"])
    Wq, bq = f(inputs["Wq"]), f(inputs["bq"])
    Wk, bk = f(inputs["Wk"]), f(inputs["bk"])
    Wv, bv = f(inputs["Wv"]), f(inputs["bv"])
    Wo, bo = f(inputs["Wo"]), f(inputs["bo"])
    in_maps = []
    for c in range(NCORES):
        b, h0 = c // 2, (c % 2) * NH
        r = slice(h0 * HD, (h0 + NH) * HD)
        in_maps.append({
            "hs": hs[b], "kv": kv[b], "bias": bias[b, h0:h0 + NH],
            "Wq": Wq[r], "Wk": Wk[r], "Wv": Wv[r], "Wo": Wo,
            "bq": bq[r], "bk": bk[r], "bv": bv[r], "bo": bo,
        })
    return in_maps


def assemble(results):
    out = np.empty((B, T, E), dtype=np.float32)
    for c in range(NCORES):
        b, h0 = c // 2, (c % 2) * NH
        out[b, h0 * HD:(h0 + NH) * HD, :] = results[c]["out"]
    return out


def kernel(**inputs):
    from concourse.bass_utils import run_bass_kernel_spmd

    nc = get_nc()
    res = run_bass_kernel_spmd(nc, make_in_maps(inputs), core_ids=list(range(NCORES)))
    return assemble(res.results)
